# revision 31
# baseline (speedup 1.0000x reference)
"""Causal self-attention (B=1, T=2048, C=1024, H=16, RoPE) on 8 TRN2 NeuronCores.

Sharding: 2 heads per core (tensor parallel on w_qkv columns / w_proj rows).
Each core computes a full-shape partial output; the host sums the 8 partials
(the tensor-parallel all-reduce, done at gather time).

Per-core pipeline:
  - host ships x already transposed (xT: C x T) so the QKV matmul needs no
    on-chip transposes of x
  - qkv = x @ w_local via fp32r matmuls (lhsT = xT tiles)
  - RoPE on q,k in natural layout; weight columns are host-permuted
    (even dims | odd dims) so rope halves are contiguous slices
  - q,k PE-transposed into (head-dim, T) layout
  - scores computed transposed: S^T[j,i] = k_j . q_i, so softmax denominators
    come free from a ones-column in V (M=65 PV matmul) and no probability
    transposes are needed; no max-subtraction (scores are O(8) here)
  - causal masking via tensor_mask_reduce on diagonal tiles only
  - normalize via reciprocal + gpsimd partition_broadcast
  - partial_out = attn_out @ w_proj_local
"""

import numpy as np

B, T, C, H = 1, 2048, 1024, 16
D = C // H  # 64
ROPE_THETA = 10000.0
N_CORES = 8
MT = T // 128  # 16 m-tiles / j-tiles
NIC = T // 512  # 4 i-chunks

_CACHE = {}


def build_module(repeat=1):
    import concourse.bass as bass
    import concourse.mybir as mybir
    import concourse.tile as tile
    from concourse import bacc
    from concourse.masks import make_identity

    f32 = mybir.dt.float32
    f32r = mybir.dt.float32r
    EXP = mybir.ActivationFunctionType.Exp
    MAX = mybir.AluOpType.max
    GE = mybir.AluOpType.is_ge

    nc = bacc.Bacc("TRN2", target_bir_lowering=False, debug=False,
                   num_devices=N_CORES)

    xT_in = nc.declare_dram_parameter("xT_in", [C, T], f32r, isOutput=False)
    w_l = nc.declare_dram_parameter("w_l", [C, 3 * 128], f32r, isOutput=False)
    wp_l = nc.declare_dram_parameter("wp_l", [128, C], f32r, isOutput=False)
    cos_t = nc.declare_dram_parameter("cos_t", [T, 32], f32, isOutput=False)
    sin_t = nc.declare_dram_parameter("sin_t", [T, 32], f32, isOutput=False)
    out_p = nc.declare_dram_parameter("out_p", [T, C], f32, isOutput=True)

    import contextlib

    with tile.TileContext(nc) as tc:
        with tc.tile_pool(name="singles", bufs=1) as singles, \
             (tc.For_i(0, repeat, 1,
                       hint_engines=(mybir.EngineType.PE,
                                     mybir.EngineType.Activation,
                                     mybir.EngineType.DVE,
                                     mybir.EngineType.Pool,
                                     mybir.EngineType.SP))
              if repeat > 1 else contextlib.nullcontext()):
            w_sb = singles.tile([128, 8, 384], f32r)
            w_r = w_l.ap().rearrange("(kt p) n -> p kt n", p=128)
            nc.sync.dma_start(out=w_sb[:, 0:4, :], in_=w_r[:, 0:4, :])
            wp_sb = singles.tile([128, 1024], f32r)
            nc.sync.dma_start(out=wp_sb, in_=wp_l[:, :])
            ident = singles.tile([128, 128], f32r)
            nc.gpsimd.memset(ident.bitcast(f32), 0.0)
            nc.gpsimd.affine_select(
                out=ident, in_=ident, compare_op=mybir.AluOpType.not_equal,
                fill=1.0, base=0, pattern=[[-1, 128]], channel_multiplier=1)

            # per-i-chunk q^T tiles, per-j-tile k^T / v tiles (fine-grained so
            # attention can start before the whole QKV phase finishes)
            qT_t = [singles.tile([128, 512], f32r, name=f"qT{i}", tag=f"qT{i}")
                    for i in range(NIC)]
            kT_t = [singles.tile([128, 128], f32r, name=f"kT{j}", tag=f"kT{j}")
                    for j in range(MT)]
            v_t = [singles.tile([128, 130], f32r, name=f"v{j}", tag=f"v{j}")
                   for j in range(MT)]
            aT_t = [singles.tile([128, 512], f32r, name=f"aT{i}", tag=f"aT{i}")
                    for i in range(NIC)]
            for j in range(MT):
                nc.vector.memset(v_t[j][:, 64:65].bitcast(f32), 1.0)
                nc.vector.memset(v_t[j][:, 129:130].bitcast(f32), 1.0)

            # x^T resident in SBUF as 8x4 tiles (128, 512); DMA'd i-chunk-major
            # so the first m-tiles unblock as early as possible.
            xT_t = [[singles.tile([128, 512], f32r, name=f"xT{k}_{c}",
                                  tag=f"xT{k}_{c}") for c in range(4)]
                    for k in range(8)]

            def dma_x_chunk(c):
                for k in range(8):
                    nc.sync.dma_start(
                        out=xT_t[k][c],
                        in_=xT_in[128 * k:128 * k + 128, 512 * c:512 * c + 512])

            for k in range(4):
                nc.sync.dma_start(
                    out=xT_t[k][0], in_=xT_in[128 * k:128 * k + 128, 0:512])
            nc.sync.dma_start(out=w_sb[:, 4:8, :], in_=w_r[:, 4:8, :])
            for k in range(4, 8):
                nc.sync.dma_start(
                    out=xT_t[k][0], in_=xT_in[128 * k:128 * k + 128, 0:512])
            cos_sb = singles.tile([128, MT, 32], f32)
            nc.sync.dma_start(out=cos_sb, in_=cos_t.ap().rearrange(
                "(mt p) d -> p mt d", p=128))
            sin_sb = singles.tile([128, MT, 32], f32)
            nc.sync.dma_start(out=sin_sb, in_=sin_t.ap().rearrange(
                "(mt p) d -> p mt d", p=128))
            for c in range(1, 4):
                dma_x_chunk(c)

            # ------------- Phase 1: QKV + RoPE + q/k transposes -------------
            with \
                 tc.tile_pool(name="qkv_ps", bufs=2, space="PSUM") as qkvpool, \
                 tc.tile_pool(name="rope_tmp", bufs=8) as tmppool, \
                 tc.tile_pool(name="rqk", bufs=6) as rqkpool, \
                 tc.tile_pool(name="big_ps", bufs=6, space="PSUM") as bigpool, \
                 tc.tile_pool(name="e_sb", bufs=10) as epool, \
                 tc.tile_pool(name="sm_sb", bufs=4) as smpool, \
                 tc.tile_pool(name="r_sb", bufs=2) as rpool, \
                 tc.tile_pool(name="rb_sb", bufs=2) as rbpool, \
                 tc.tile_pool(name="o_out", bufs=6) as ospool:
                def emit_qkv(m):
                    qkv_ps = qkvpool.tile([128, 384], f32, name=f"qkv{m}",
                                          tag="qkv")
                    c, mo = m // 4, m % 4
                    for k in range(8):
                        nc.tensor.matmul(
                            qkv_ps,
                            xT_t[k][c][:, 128 * mo:128 * mo + 128],
                            w_sb[:, k, :],
                            start=(k == 0), stop=(k == 7))

                    # RoPE: DVE multiplies all of q,k by cos and by sin in
                    # two 256-wide ops (table broadcast via stride-0 dim);
                    # gpsimd combines halves into the rotated result.
                    cos_b = bass.AP(tensor=cos_sb.tensor,
                                    offset=cos_sb[:, m, :].offset,
                                    ap=[cos_sb.ap[0], [0, 8], [1, 32]])
                    sin_b = bass.AP(tensor=sin_sb.tensor,
                                    offset=sin_sb[:, m, :].offset,
                                    ap=[sin_sb.ap[0], [0, 8], [1, 32]])
                    src8 = qkv_ps[:, 0:256].rearrange("p (b d) -> p b d", b=8)
                    tcos = tmppool.tile([128, 8, 32], f32, name=f"tc_{m}",
                                        tag="tc")
                    tsin = tmppool.tile([128, 8, 32], f32, name=f"ts_{m}",
                                        tag="ts")
                    nc.vector.tensor_mul(tcos, src8, cos_b)
                    nc.vector.tensor_mul(tsin, src8, sin_b)
                    rqk = rqkpool.tile([128, 4, 64], f32r, name=f"rqk{m}",
                                       tag="rqk")
                    tc4 = tcos.rearrange("p (b two) d -> p b two d", two=2)
                    ts4 = tsin.rearrange("p (b two) d -> p b two d", two=2)
                    nc.gpsimd.tensor_sub(rqk[:, :, 0:32],
                                         tc4[:, :, 0, :], ts4[:, :, 1, :])
                    nc.gpsimd.tensor_add(rqk[:, :, 32:64],
                                         ts4[:, :, 0, :], tc4[:, :, 1, :])

                    nc.vector.tensor_copy(v_t[m][:, 0:64], qkv_ps[:, 256:320])
                    nc.vector.tensor_copy(v_t[m][:, 65:129], qkv_ps[:, 320:384])

                    # transpose rope'd q,k back into (dim, T)
                    rqk2 = rqk.rearrange("p b d -> p (b d)")
                    tp_ps = bigpool.tile([128, 256], f32r, name=f"tp{m}",
                                         tag="big")
                    nc.tensor.transpose(tp_ps[:, 0:128], rqk2[:, 0:128], ident)
                    nc.tensor.transpose(tp_ps[:, 128:256], rqk2[:, 128:256],
                                        ident)
                    nc.scalar.copy(qT_t[m // 4][:, 128 * (m % 4):128 * (m % 4) + 128],
                                   tp_ps[:, 0:128])
                    nc.scalar.copy(kT_t[m], tp_ps[:, 128:256])

                def emit_attn(ic):
                    J = 4 * ic + 4
                    o_ps = [bigpool.tile([128, 512], f32, name=f"o{ic}_{h}",
                                         tag="big") for h in range(2)]
                    e_tiles = {}

                    def emit_S(jt, ic=ic, e_tiles=e_tiles):
                        mm = jt - 4 * ic
                        for h in range(2):
                            lhsT = kT_t[jt][64 * h:64 * h + 64, :]
                            if mm < 0:
                                s_ps = bigpool.tile([128, 512], f32,
                                                    name=f"s{ic}_{jt}_{h}",
                                                    tag="big")
                                nc.tensor.matmul(
                                    s_ps, lhsT, qT_t[ic][64 * h:64 * h + 64, :],
                                    start=True, stop=True)
                                e_t = epool.tile([128, 512], f32r,
                                                 name=f"e{ic}_{jt}_{h}", tag="e")
                                nc.scalar.activation(e_t, s_ps, EXP, scale=0.125)
                                e_tiles[(h, jt)] = (e_t, 512)
                            else:
                                L = 512 - 128 * mm
                                s_ps = bigpool.tile([128, 512], f32,
                                                    name=f"s{ic}_{jt}_{h}",
                                                    tag="big")
                                nc.tensor.matmul(
                                    s_ps[:, 0:L], lhsT,
                                    qT_t[ic][64 * h:64 * h + 64, 128 * mm:512],
                                    start=True, stop=True)
                                e_t = epool.tile([128, 512], f32r,
                                                 name=f"e{ic}_{jt}_{h}", tag="e")
                                nc.scalar.activation(e_t[:, 0:L], s_ps[:, 0:L],
                                                     EXP, scale=0.125)
                                # zero the causally-masked region: keep where
                                # (free pos) - (partition jj) >= 0
                                nc.gpsimd.affine_select(
                                    out=e_t[:, 0:L], in_=e_t[:, 0:L],
                                    compare_op=GE, fill=0.0, base=0,
                                    pattern=[[1, L]], channel_multiplier=-1)
                                e_tiles[(h, jt)] = (e_t, L)

                    def emit_PV(jt, ic=ic, J=J, e_tiles=e_tiles, o_ps=o_ps):
                        for h in range(2):
                            e_t, L = e_tiles.pop((h, jt))
                            lhsT = v_t[jt][:, 65 * h:65 * h + 65]
                            out_ap = (o_ps[h][0:65, :] if L == 512
                                      else o_ps[h][0:65, 512 - L:512])
                            nc.tensor.matmul(
                                out_ap, lhsT, e_t[:, 0:L],
                                start=(jt == 0), stop=(jt == J - 1),
                                skip_group_check=True)

                    depth = 2 if J > 2 else 1
                    for jt in range(J):
                        emit_S(jt)
                        if jt >= depth:
                            emit_PV(jt - depth)
                    for jt in range(J - depth, J):
                        emit_PV(jt)

                    for h in range(2):
                        r_t = rpool.tile([1, 512], f32)
                        nc.vector.reciprocal(r_t, o_ps[h][64:65, :])
                        rb_t = rbpool.tile([64, 512], f32)
                        nc.gpsimd.partition_broadcast(rb_t, r_t[0:1, :],
                                                      channels=64)
                        nc.vector.tensor_mul(
                            aT_t[ic][64 * h:64 * h + 64, :],
                            o_ps[h][0:64, :], rb_t)

                def emit_proj(ic):
                    for m in range(4 * ic, 4 * ic + 4):
                        o_sb = ospool.tile([128, 1024], f32, name=f"os{m}",
                                           tag="os")
                        for n2 in range(2):
                            p_ps = bigpool.tile([128, 512], f32,
                                                name=f"p{m}_{n2}", tag="big")
                            nc.tensor.matmul(
                                p_ps,
                                aT_t[m // 4][:, 128 * (m % 4):128 * (m % 4) + 128],
                                wp_sb[:, 512 * n2:512 * n2 + 512],
                                start=True, stop=True)
                            if n2 == 0:
                                nc.scalar.copy(o_sb[:, 0:512], p_ps)
                            else:
                                nc.vector.tensor_copy(o_sb[:, 512:1024], p_ps)
                        nc.sync.dma_start(
                            out=out_p[128 * m:128 * m + 128, :], in_=o_sb)

                # Software-pipelined emission: QKV runs half a chunk ahead
                # of attention so attention matmuls cover the rope chain
                # latency, and the next-chunk QKV covers the normalize chain
                # before each projection.
                for m in range(4):
                    emit_qkv(m)
                for ic in range(NIC):
                    emit_attn(ic)
                    if ic + 1 < NIC:
                        for m in range(4 * ic + 4, 4 * ic + 8):
                            emit_qkv(m)
                    emit_proj(ic)

    nc.compile()
    return nc


def host_inputs(x, w_qkv, w_proj):
    """Build per-core input maps from the full inputs."""
    x2 = np.asarray(x, dtype=np.float32).reshape(T, C)
    xT = np.ascontiguousarray(x2.T)
    wq = np.asarray(w_qkv, dtype=np.float32)
    wp = np.asarray(w_proj, dtype=np.float32)

    inv_freq = 1.0 / (ROPE_THETA ** (np.arange(0, D, 2, dtype=np.float32) / D))
    ang = np.arange(T, dtype=np.float32)[:, None] * inv_freq[None, :]
    cos32 = np.cos(ang).astype(np.float32)   # (T, 32)
    sin32 = np.sin(ang).astype(np.float32)
    cos_t = cos32
    sin_t = sin32

    perm = np.concatenate([np.arange(0, D, 2), np.arange(1, D, 2)])  # evens|odds

    in_maps = []
    for c in range(N_CORES):
        h0, h1 = 2 * c, 2 * c + 1
        cols = []
        for h in (h0, h1):      # q blocks, permuted
            cols.append(wq[:, h * D:(h + 1) * D][:, perm])
        for h in (h0, h1):      # k blocks, permuted
            cols.append(wq[:, C + h * D:C + (h + 1) * D][:, perm])
        for h in (h0, h1):      # v blocks, natural
            cols.append(wq[:, 2 * C + h * D:2 * C + (h + 1) * D])
        w_l = np.ascontiguousarray(np.concatenate(cols, axis=1))  # (1024, 384)
        wp_l = np.ascontiguousarray(wp[128 * c:128 * c + 128, :])  # (128, 1024)
        in_maps.append({
            "xT_in": xT, "w_l": w_l, "wp_l": wp_l,
            "cos_t": cos_t, "sin_t": sin_t,
        })
    return in_maps


def kernel(x, w_qkv, w_proj):
    from concourse.bass_utils import run_bass_kernel_spmd

    if "nc" not in _CACHE:
        _CACHE["nc"] = build_module()
    nc = _CACHE["nc"]

    in_maps = host_inputs(x, w_qkv, w_proj)
    res = run_bass_kernel_spmd(nc, in_maps, list(range(N_CORES)))
    out = np.zeros((T, C), dtype=np.float64)
    for c in range(N_CORES):
        out += res.results[c]["out_p"].astype(np.float64)
    return out.astype(np.float32).reshape(B, T, C)


# revision 32
# speedup vs baseline: 6.8766x; 6.8766x over previous
"""Causal self-attention (B=1, T=2048, C=1024, H=16, RoPE) on 8 TRN2 NeuronCores.

Sharding: 2 heads per core (tensor parallel on w_qkv columns / w_proj rows).
Each core computes a full-shape partial output; the host sums the 8 partials
(the tensor-parallel all-reduce, done at gather time).

Per-core pipeline:
  - host ships x already transposed (xT: C x T) so the QKV matmul needs no
    on-chip transposes of x
  - qkv = x @ w_local via fp32r matmuls (lhsT = xT tiles)
  - RoPE on q,k in natural layout; weight columns are host-permuted
    (even dims | odd dims) so rope halves are contiguous slices
  - q,k PE-transposed into (head-dim, T) layout
  - scores computed transposed: S^T[j,i] = k_j . q_i, so softmax denominators
    come free from a ones-column in V (M=65 PV matmul) and no probability
    transposes are needed; no max-subtraction (scores are O(8) here)
  - causal masking via tensor_mask_reduce on diagonal tiles only
  - normalize via reciprocal + gpsimd partition_broadcast
  - partial_out = attn_out @ w_proj_local
"""

import numpy as np

B, T, C, H = 1, 2048, 1024, 16
D = C // H  # 64
ROPE_THETA = 10000.0
N_CORES = 8
MT = T // 128  # 16 m-tiles / j-tiles
NIC = T // 512  # 4 i-chunks

_CACHE = {}


def build_module(repeat=1):
    import concourse.bass as bass
    import concourse.mybir as mybir
    import concourse.tile as tile
    from concourse import bacc
    from concourse.masks import make_identity

    f32 = mybir.dt.float32
    f32r = mybir.dt.float32r
    EXP = mybir.ActivationFunctionType.Exp
    MAX = mybir.AluOpType.max
    GE = mybir.AluOpType.is_ge

    nc = bacc.Bacc("TRN2", target_bir_lowering=False, debug=False,
                   num_devices=N_CORES)

    xT_in = nc.declare_dram_parameter("xT_in", [C, T], f32r, isOutput=False)
    w_l = nc.declare_dram_parameter("w_l", [C, 3 * 128], f32r, isOutput=False)
    wp_l = nc.declare_dram_parameter("wp_l", [128, C], f32r, isOutput=False)
    cos_t = nc.declare_dram_parameter("cos_t", [T, 32], f32, isOutput=False)
    sin_t = nc.declare_dram_parameter("sin_t", [T, 32], f32, isOutput=False)
    out_p = nc.declare_dram_parameter("out_p", [T, C], f32, isOutput=True)

    import contextlib

    with tile.TileContext(nc) as tc:
        with tc.tile_pool(name="singles", bufs=1) as singles, \
             (tc.For_i(0, repeat, 1,
                       hint_engines=(mybir.EngineType.PE,
                                     mybir.EngineType.Activation,
                                     mybir.EngineType.DVE,
                                     mybir.EngineType.Pool,
                                     mybir.EngineType.SP))
              if repeat > 1 else contextlib.nullcontext()):
            w_sb = singles.tile([128, 8, 384], f32r)
            w_r = w_l.ap().rearrange("(kt p) n -> p kt n", p=128)
            nc.sync.dma_start(out=w_sb[:, 0:4, :], in_=w_r[:, 0:4, :])
            wp_sb = singles.tile([128, 1024], f32r)
            nc.sync.dma_start(out=wp_sb, in_=wp_l[:, :])
            ident = singles.tile([128, 128], f32r)
            nc.gpsimd.memset(ident.bitcast(f32), 0.0)
            nc.gpsimd.affine_select(
                out=ident, in_=ident, compare_op=mybir.AluOpType.not_equal,
                fill=1.0, base=0, pattern=[[-1, 128]], channel_multiplier=1)

            # per-i-chunk q^T tiles, per-j-tile k^T / v tiles (fine-grained so
            # attention can start before the whole QKV phase finishes)
            qT_t = [singles.tile([128, 512], f32r, name=f"qT{i}", tag=f"qT{i}")
                    for i in range(NIC)]
            kT_t = [singles.tile([128, 128], f32r, name=f"kT{j}", tag=f"kT{j}")
                    for j in range(MT)]
            v_t = [singles.tile([128, 130], f32r, name=f"v{j}", tag=f"v{j}")
                   for j in range(MT)]
            aT_t = [singles.tile([128, 512], f32r, name=f"aT{i}", tag=f"aT{i}")
                    for i in range(NIC)]
            for j in range(MT):
                nc.vector.memset(v_t[j][:, 64:65].bitcast(f32), 1.0)
                nc.vector.memset(v_t[j][:, 129:130].bitcast(f32), 1.0)

            # x^T resident in SBUF as 8x4 tiles (128, 512); DMA'd i-chunk-major
            # so the first m-tiles unblock as early as possible.
            xT_t = [[singles.tile([128, 512], f32r, name=f"xT{k}_{c}",
                                  tag=f"xT{k}_{c}") for c in range(4)]
                    for k in range(8)]

            def dma_x_chunk(c):
                for k in range(8):
                    nc.sync.dma_start(
                        out=xT_t[k][c],
                        in_=xT_in[128 * k:128 * k + 128, 512 * c:512 * c + 512])

            for k in range(4):
                nc.sync.dma_start(
                    out=xT_t[k][0], in_=xT_in[128 * k:128 * k + 128, 0:512])
            nc.sync.dma_start(out=w_sb[:, 4:8, :], in_=w_r[:, 4:8, :])
            for k in range(4, 8):
                nc.sync.dma_start(
                    out=xT_t[k][0], in_=xT_in[128 * k:128 * k + 128, 0:512])
            cos_sb = singles.tile([128, MT, 32], f32)
            nc.sync.dma_start(out=cos_sb, in_=cos_t.ap().rearrange(
                "(mt p) d -> p mt d", p=128))
            sin_sb = singles.tile([128, MT, 32], f32)
            nc.sync.dma_start(out=sin_sb, in_=sin_t.ap().rearrange(
                "(mt p) d -> p mt d", p=128))
            for c in range(1, 4):
                dma_x_chunk(c)

            # ------------- Phase 1: QKV + RoPE + q/k transposes -------------
            with \
                 tc.tile_pool(name="qkv_ps", bufs=2, space="PSUM") as qkvpool, \
                 tc.tile_pool(name="rope_tmp", bufs=4) as tmppool, \
                 tc.tile_pool(name="rqk", bufs=4) as rqkpool, \
                 tc.tile_pool(name="big_ps", bufs=6, space="PSUM") as bigpool, \
                 tc.tile_pool(name="e_sb", bufs=8) as epool, \
                 tc.tile_pool(name="sm_sb", bufs=4) as smpool, \
                 tc.tile_pool(name="r_sb", bufs=2) as rpool, \
                 tc.tile_pool(name="rb_sb", bufs=2) as rbpool, \
                 tc.tile_pool(name="o_out", bufs=4) as ospool:
                def emit_qkv(m):
                    qkv_ps = qkvpool.tile([128, 384], f32, name=f"qkv{m}",
                                          tag="qkv")
                    c, mo = m // 4, m % 4
                    for k in range(8):
                        nc.tensor.matmul(
                            qkv_ps,
                            xT_t[k][c][:, 128 * mo:128 * mo + 128],
                            w_sb[:, k, :],
                            start=(k == 0), stop=(k == 7))

                    # RoPE: DVE multiplies all of q,k by cos and by sin in
                    # two 256-wide ops (table broadcast via stride-0 dim);
                    # gpsimd combines halves into the rotated result.
                    cos_b = bass.AP(tensor=cos_sb.tensor,
                                    offset=cos_sb[:, m, :].offset,
                                    ap=[cos_sb.ap[0], [0, 8], [1, 32]])
                    sin_b = bass.AP(tensor=sin_sb.tensor,
                                    offset=sin_sb[:, m, :].offset,
                                    ap=[sin_sb.ap[0], [0, 8], [1, 32]])
                    src8 = qkv_ps[:, 0:256].rearrange("p (b d) -> p b d", b=8)
                    tcos = tmppool.tile([128, 8, 32], f32, name=f"tc_{m}",
                                        tag="tc")
                    tsin = tmppool.tile([128, 8, 32], f32, name=f"ts_{m}",
                                        tag="ts")
                    nc.vector.tensor_mul(tcos, src8, cos_b)
                    nc.vector.tensor_mul(tsin, src8, sin_b)
                    rqk = rqkpool.tile([128, 4, 64], f32r, name=f"rqk{m}",
                                       tag="rqk")
                    tc4 = tcos.rearrange("p (b two) d -> p b two d", two=2)
                    ts4 = tsin.rearrange("p (b two) d -> p b two d", two=2)
                    nc.gpsimd.tensor_sub(rqk[:, :, 0:32],
                                         tc4[:, :, 0, :], ts4[:, :, 1, :])
                    nc.gpsimd.tensor_add(rqk[:, :, 32:64],
                                         ts4[:, :, 0, :], tc4[:, :, 1, :])

                    nc.vector.tensor_copy(v_t[m][:, 0:64], qkv_ps[:, 256:320])
                    nc.vector.tensor_copy(v_t[m][:, 65:129], qkv_ps[:, 320:384])

                    # transpose rope'd q,k back into (dim, T)
                    rqk2 = rqk.rearrange("p b d -> p (b d)")
                    tp_ps = bigpool.tile([128, 256], f32r, name=f"tp{m}",
                                         tag="big")
                    nc.tensor.transpose(tp_ps[:, 0:128], rqk2[:, 0:128], ident)
                    nc.tensor.transpose(tp_ps[:, 128:256], rqk2[:, 128:256],
                                        ident)
                    nc.scalar.copy(qT_t[m // 4][:, 128 * (m % 4):128 * (m % 4) + 128],
                                   tp_ps[:, 0:128])
                    nc.scalar.copy(kT_t[m], tp_ps[:, 128:256])

                def emit_attn(ic):
                    J = 4 * ic + 4
                    o_ps = [bigpool.tile([128, 512], f32, name=f"o{ic}_{h}",
                                         tag="big") for h in range(2)]
                    e_tiles = {}

                    def emit_S(jt, ic=ic, e_tiles=e_tiles):
                        mm = jt - 4 * ic
                        for h in range(2):
                            lhsT = kT_t[jt][64 * h:64 * h + 64, :]
                            if mm < 0:
                                s_ps = bigpool.tile([128, 512], f32,
                                                    name=f"s{ic}_{jt}_{h}",
                                                    tag="big")
                                nc.tensor.matmul(
                                    s_ps, lhsT, qT_t[ic][64 * h:64 * h + 64, :],
                                    start=True, stop=True)
                                e_t = epool.tile([128, 512], f32r,
                                                 name=f"e{ic}_{jt}_{h}", tag="e")
                                nc.scalar.activation(e_t, s_ps, EXP, scale=0.125)
                                e_tiles[(h, jt)] = (e_t, 512)
                            else:
                                L = 512 - 128 * mm
                                s_ps = bigpool.tile([128, 512], f32,
                                                    name=f"s{ic}_{jt}_{h}",
                                                    tag="big")
                                nc.tensor.matmul(
                                    s_ps[:, 0:L], lhsT,
                                    qT_t[ic][64 * h:64 * h + 64, 128 * mm:512],
                                    start=True, stop=True)
                                e_t = epool.tile([128, 512], f32r,
                                                 name=f"e{ic}_{jt}_{h}", tag="e")
                                nc.scalar.activation(e_t[:, 0:L], s_ps[:, 0:L],
                                                     EXP, scale=0.125)
                                # zero the causally-masked region: keep where
                                # (free pos) - (partition jj) >= 0
                                nc.gpsimd.affine_select(
                                    out=e_t[:, 0:L], in_=e_t[:, 0:L],
                                    compare_op=GE, fill=0.0, base=0,
                                    pattern=[[1, L]], channel_multiplier=-1)
                                e_tiles[(h, jt)] = (e_t, L)

                    def emit_PV(jt, ic=ic, J=J, e_tiles=e_tiles, o_ps=o_ps):
                        for h in range(2):
                            e_t, L = e_tiles.pop((h, jt))
                            lhsT = v_t[jt][:, 65 * h:65 * h + 65]
                            out_ap = (o_ps[h][0:65, :] if L == 512
                                      else o_ps[h][0:65, 512 - L:512])
                            nc.tensor.matmul(
                                out_ap, lhsT, e_t[:, 0:L],
                                start=(jt == 0), stop=(jt == J - 1),
                                skip_group_check=True)

                    depth = 2 if J > 2 else 1
                    for jt in range(J):
                        emit_S(jt)
                        if jt >= depth:
                            emit_PV(jt - depth)
                    for jt in range(J - depth, J):
                        emit_PV(jt)

                    for h in range(2):
                        r_t = rpool.tile([1, 512], f32)
                        nc.vector.reciprocal(r_t, o_ps[h][64:65, :])
                        rb_t = rbpool.tile([64, 512], f32)
                        nc.gpsimd.partition_broadcast(rb_t, r_t[0:1, :],
                                                      channels=64)
                        nc.vector.tensor_mul(
                            aT_t[ic][64 * h:64 * h + 64, :],
                            o_ps[h][0:64, :], rb_t)

                def emit_proj(ic):
                    for m in range(4 * ic, 4 * ic + 4):
                        o_sb = ospool.tile([128, 1024], f32, name=f"os{m}",
                                           tag="os")
                        for n2 in range(2):
                            p_ps = bigpool.tile([128, 512], f32,
                                                name=f"p{m}_{n2}", tag="big")
                            nc.tensor.matmul(
                                p_ps,
                                aT_t[m // 4][:, 128 * (m % 4):128 * (m % 4) + 128],
                                wp_sb[:, 512 * n2:512 * n2 + 512],
                                start=True, stop=True)
                            if n2 == 0:
                                nc.scalar.copy(o_sb[:, 0:512], p_ps)
                            else:
                                nc.vector.tensor_copy(o_sb[:, 512:1024], p_ps)
                        nc.sync.dma_start(
                            out=out_p[128 * m:128 * m + 128, :], in_=o_sb)

                # Software-pipelined emission: QKV runs half a chunk ahead
                # of attention so attention matmuls cover the rope chain
                # latency, and the next-chunk QKV covers the normalize chain
                # before each projection.
                for m in range(4):
                    emit_qkv(m)
                for ic in range(NIC):
                    emit_attn(ic)
                    if ic + 1 < NIC:
                        for m in range(4 * ic + 4, 4 * ic + 8):
                            emit_qkv(m)
                    emit_proj(ic)

    nc.compile()
    return nc


def host_inputs(x, w_qkv, w_proj):
    """Build per-core input maps from the full inputs."""
    x2 = np.asarray(x, dtype=np.float32).reshape(T, C)
    xT = np.ascontiguousarray(x2.T)
    wq = np.asarray(w_qkv, dtype=np.float32)
    wp = np.asarray(w_proj, dtype=np.float32)

    inv_freq = 1.0 / (ROPE_THETA ** (np.arange(0, D, 2, dtype=np.float32) / D))
    ang = np.arange(T, dtype=np.float32)[:, None] * inv_freq[None, :]
    cos32 = np.cos(ang).astype(np.float32)   # (T, 32)
    sin32 = np.sin(ang).astype(np.float32)
    cos_t = cos32
    sin_t = sin32

    perm = np.concatenate([np.arange(0, D, 2), np.arange(1, D, 2)])  # evens|odds

    in_maps = []
    for c in range(N_CORES):
        h0, h1 = 2 * c, 2 * c + 1
        cols = []
        for h in (h0, h1):      # q blocks, permuted
            cols.append(wq[:, h * D:(h + 1) * D][:, perm])
        for h in (h0, h1):      # k blocks, permuted
            cols.append(wq[:, C + h * D:C + (h + 1) * D][:, perm])
        for h in (h0, h1):      # v blocks, natural
            cols.append(wq[:, 2 * C + h * D:2 * C + (h + 1) * D])
        w_l = np.ascontiguousarray(np.concatenate(cols, axis=1))  # (1024, 384)
        wp_l = np.ascontiguousarray(wp[128 * c:128 * c + 128, :])  # (128, 1024)
        in_maps.append({
            "xT_in": xT, "w_l": w_l, "wp_l": wp_l,
            "cos_t": cos_t, "sin_t": sin_t,
        })
    return in_maps


def kernel(x, w_qkv, w_proj):
    from concourse.bass_utils import run_bass_kernel_spmd

    if "nc" not in _CACHE:
        _CACHE["nc"] = build_module()
    nc = _CACHE["nc"]

    in_maps = host_inputs(x, w_qkv, w_proj)
    res = run_bass_kernel_spmd(nc, in_maps, list(range(N_CORES)))
    out = np.zeros((T, C), dtype=np.float64)
    for c in range(N_CORES):
        out += res.results[c]["out_p"].astype(np.float64)
    return out.astype(np.float32).reshape(B, T, C)
